# revision 2
# baseline (speedup 1.0000x reference)
"""Chamfer distance (L2, squared) Bass kernel for Trainium2.

Problem: xyz1 (4, 8192, 3), xyz2 (4, 8192, 3) float32.
  d2[b, n, m] = ||xyz1[b,n] - xyz2[b,m]||^2
  out = mean_n(min_m d2) + mean_m(min_n d2)   (scalar, float32)

Sharding: 8 cores = (batch b in 0..3) x (half h of the N axis). Each core
computes, for its (b, h):
  - dist1[n] = min over all M of d2 for its 4096 rows (complete), and
  - partial dist2[m] = min over its 4096 rows (combined across the 2
    halves on the host with an elementwise min).

Device algorithm (per core):
  The PE emits NEGATED squared distances via a single K=16 bf16 matmul
  per tile using an exact-ish hi/lo decomposition
  (x = bf16(x) + bf16(x - bf16(x))):
    -d2 = (2 x1) . x2 - ||x1||^2 - ||x2||^2
  with features
    F1 = [y1h y1h y1l y1l n1h n1l 1 1]   (y1 = 2 x1, n1* = -||x1||^2 hi/lo)
    F2 = [x2h x2l x2h x2l 1 1 n2h n2l]   (n2* = -||x2||^2 hi/lo)
  so F1 . F2 accumulates all four cross products plus both (negated)
  norms in fp32 PSUM. The same two feature buffers serve as stationary
  or moving for the two output orientations (A: m on partitions ->
  dist2; B: n on partitions -> dist1).

  Reduction (min d2 == -max(-d2)): PSUM tiles are consumed in PAIRS by
  a runtime-registered custom DVE op PAIRMAX_ANT:
    out = max(Src0, Src1) elementwise; accum_out = max(seed, max(out))
  ScalarE evacuates tile A of each pair to SBUF fp16; the DVE PAIRMAX
  then streams tile B straight from PSUM (fp32) against the fp16 copy,
  max-accumulating into a [128,1] column with seed chaining across
  pairs. One DVE instruction consumes two tiles; there are no
  fold/merge chains and no epilogue reduce. Steady state per pair:
  PE 1.7us, ScalarE 2.0us, DVE 2.3us -> DVE-bound.

  fp16 only rounds tile-A values (~2.4e-4 rel, unbiased); measured
  output error vs the fp32 reference is ~1e-5..2e-4.
"""

import numpy as np

import concourse.bass as bass
import concourse.tile as tile
from concourse import bacc, mybir
from concourse.bass_utils import run_bass_kernel_spmd

B, N, M = 4, 8192, 8192
NCORES = 8
NHALF = N // 2  # 4096 xyz1 rows per core

F32 = mybir.dt.float32
BF16 = mybir.dt.bfloat16
FP16 = mybir.dt.float16
BIG = 3.0e38

MT2 = M // 128  # 64 stationary chunks, orientation A
NT1 = NHALF // 128  # 32 stationary chunks, orientation B
TA = NHALF // 2048  # 2 moving tiles of 2048, orientation A
TB = M // 2048  # 4 moving tiles of 2048, orientation B

MULT = mybir.AluOpType.mult
SUB = mybir.AluOpType.subtract
BYP = mybir.AluOpType.bypass


def register_pairmax():
    """Register the PAIRMAX_ANT custom DVE op (idempotent).

    out = max(Src0, Src1); accum_out = max(accum_init(=s0), max(out)).
    uops_sha is computed at registration so the pinned-sha check in
    DveOp.compile passes without editing concourse.
    """
    from concourse.dve_spec import Spec, Src0, Src1, C0, maxx, lower
    from concourse.dve_uop import DveOpSpec
    from concourse import dve_ops
    from concourse.dve_ops import DveOp

    if "PAIRMAX_ANT" in dve_ops._SUB_OPCODE_FOR_NAME:
        return next(o for o in dve_ops.OPS if o.name == "PAIRMAX_ANT")

    spec = Spec(
        body=maxx(Src0, Src1),
        accum=maxx,
        accum_init=C0,
        reference=lambda in0, in1, s0, s1, imm2: (
            np.maximum(in0, in1),
            np.maximum(np.maximum(in0, in1).max(axis=-1), s0.reshape(-1)),
        ),
    )
    row = dve_ops._CUSTOM_DVE_ROW_BASE + len(dve_ops.OPS)
    shas = {}
    for ver in ("v3", "v4"):
        uops = lower(spec, ver=ver)
        shas[ver] = DveOpSpec(
            name="PAIRMAX_ANT", opcode=row, uops=uops, rd1_en=True
        ).sha(ver)
    op = DveOp("PAIRMAX_ANT", spec, subdim=False, uops_sha=shas)
    dve_ops.OPS.append(op)
    dve_ops._SUB_OPCODE_FOR_NAME[op.name] = row
    dve_ops.CUSTOM_DVE_SPECS[op.name] = op.spec
    return op


PAIRMAX = register_pairmax()


def _build_body(tc, x1t, x2t, dist1, dist2p, repeat):
    # Compute instructions may only start at partition 0/32/64/96 (BIR
    # verifier rule), so all feature rows are computed at partition base 0
    # and placed into their final partition rows via SBUF->SBUF DMAs
    # (DMAs are exempt from the rule).
    nc = tc.nc
    stt = nc.vector.scalar_tensor_tensor

    persist = tc.alloc_tile_pool(name="persist", bufs=1)
    prep = tc.alloc_tile_pool(name="prep", bufs=1)

    f1 = persist.tile([16, NHALF], BF16)
    f2 = persist.tile([16, M], BF16)
    d1col = persist.tile([128, NT1], F32)  # negated: max(-d2) per chunk
    d2col = persist.tile([128, MT2], F32)

    neg_st = persist.tile([3, 128], F32)
    nc.vector.memset(neg_st[:], -1.0)
    nc.gpsimd.memset(f1[:], 1.0)
    nc.gpsimd.memset(f2[:], 1.0)

    def build_features(xin, feat, width, scale, hi_dup_rows, lo_rows, nrm_rows):
        # Shared-tag scratch so the x1 and x2 phases reuse the same SBUF.
        st = prep.tile([3, width], F32, tag="st", name="st")
        sq = prep.tile([3, width], F32, tag="sq", name="sq")
        lo = prep.tile([3, width], BF16, tag="lo", name="lo")
        nh = prep.tile([1, width], BF16, tag="nh", name="nh")
        nl = prep.tile([1, width], BF16, tag="nl", name="nl")

        nc.sync.dma_start(st[:], xin.ap())
        # -||x||^2: fp32 squares (ScalarE), then a (-1)-stationary matmul
        # broadcasts the negated per-point norm onto all 128 PSUM
        # partitions; row 0 is split hi/lo straight out of PSUM.
        nc.scalar.activation(sq[:], st[:], mybir.ActivationFunctionType.Square)
        psn = tc.alloc_tile_pool(name="psn", bufs=2, space="PSUM")
        for c in range(width // 512):
            sl = slice(512 * c, 512 * (c + 1))
            pn = psn.tile([128, 512], F32, tag="pn", name="pn")
            nc.tensor.matmul(pn[:], neg_st[:], sq[:, sl], start=True, stop=True)
            nc.scalar.copy(nh[0:1, sl], pn[0:1, :])
            stt(nl[0:1, sl], pn[0:1, :], 0.0, nh[0:1, sl], BYP, SUB)
        psn.release()

        # hi/lo split of (scale * x) at partition base 0.
        nc.scalar.mul(feat[0:3, :], st[:], scale)  # hi -> rows 0-2
        stt(lo[:], st[:], scale, feat[0:3, :], MULT, SUB)

        # Place remaining rows (DMAs may start at any partition).
        nc.sync.dma_start(feat[hi_dup_rows[0] : hi_dup_rows[0] + 3, :], feat[0:3, :])
        for r in lo_rows:
            nc.sync.dma_start(feat[r : r + 3, :], lo[:])
        nc.sync.dma_start(feat[nrm_rows[0] : nrm_rows[0] + 1, :], nh[:])
        nc.sync.dma_start(feat[nrm_rows[1] : nrm_rows[1] + 1, :], nl[:])

    # F1 = [y1h y1h y1l y1l n1h n1l 1 1]   (y1 = 2 x1)
    build_features(x1t, f1, NHALF, 2.0, (3,), (6, 9), (12, 13))
    # F2 = [x2h x2l x2h x2l 1 1 n2h n2l]
    build_features(x2t, f2, M, 1.0, (6,), (3, 9), (14, 15))

    prep.release()

    aux = tc.alloc_tile_pool(name="aux", bufs=1)
    ps_pool = tc.alloc_tile_pool(name="ps_pool", bufs=2, space="PSUM")

    def sweep(stat, mov, n_stat, n_tiles, outcols):
        # One output orientation: for each 128-wide stationary chunk,
        # stream moving tiles of 2048 in pairs. Tile A of each pair is
        # evacuated to SBUF fp16 by ScalarE; tile B is consumed by one
        # PAIRMAX straight from PSUM, max-accumulating with seed
        # chaining. The last pair's accum_out lands in outcols[:, s].
        for s in range(n_stat):
            seed = -BIG
            for p in range(n_tiles // 2):
                psA = ps_pool.tile([128, 2048], F32, tag="ps", name="ps")
                for j in range(4):
                    c = (2 * p) * 4 + j
                    nc.tensor.matmul(
                        psA[:, 512 * j : 512 * (j + 1)],
                        stat[:, 128 * s : 128 * (s + 1)],
                        mov[:, 512 * c : 512 * (c + 1)],
                        start=True,
                        stop=True,
                    )
                gt = aux.tile([128, 2048], FP16, tag="gt", name="gt", bufs=3)
                nc.scalar.copy(gt[:], psA[:])
                psB = ps_pool.tile([128, 2048], F32, tag="ps", name="ps")
                for j in range(4):
                    c = (2 * p + 1) * 4 + j
                    nc.tensor.matmul(
                        psB[:, 512 * j : 512 * (j + 1)],
                        stat[:, 128 * s : 128 * (s + 1)],
                        mov[:, 512 * c : 512 * (c + 1)],
                        start=True,
                        stop=True,
                    )
                last = p == n_tiles // 2 - 1
                sc = aux.tile([128, 2048], FP16, tag="sc", name="sc", bufs=2)
                if last:
                    accout = outcols[:, s : s + 1]
                else:
                    accout = aux.tile([128, 1], F32, tag="seed", name="seed", bufs=2)
                nc.vector._custom_dve(
                    PAIRMAX,
                    out=sc[:],
                    in0=psB[:],
                    in1=gt[:],
                    s0=seed,
                    accum_out=accout,
                )
                seed = accout[:]

    def one_pass():
        sweep(f2, f1, MT2, TA, d2col)  # A: m on partitions -> dist2
        sweep(f1, f2, NT1, TB, d1col)  # B: n on partitions -> dist1

    if repeat == 1:
        one_pass()
    else:
        # Benchmarking mode: re-run the main loop on-device so its cost
        # dominates the fixed host/RPC dispatch overhead.
        with tc.For_i(0, repeat, 1):
            one_pass()

    ps_pool.release()
    aux.release()

    # Clamp: d2 >= 0  <=>  max(-d2) <= 0.
    nc.vector.tensor_scalar_min(d1col[:], d1col[:], 0.0)
    nc.vector.tensor_scalar_min(d2col[:], d2col[:], 0.0)
    nc.sync.dma_start(dist1.ap(), d1col[:])
    nc.sync.dma_start(dist2p.ap(), d2col[:])

    persist.release()


def build_nc(repeat=1):
    nc = bacc.Bacc(
        "TRN2", target_bir_lowering=False, debug=False, num_devices=NCORES
    )
    x1t = nc.dram_tensor("x1t", [3, NHALF], F32, kind="ExternalInput")
    x2t = nc.dram_tensor("x2t", [3, M], F32, kind="ExternalInput")
    dist1 = nc.dram_tensor("dist1", [128, NT1], F32, kind="ExternalOutput")
    dist2p = nc.dram_tensor("dist2p", [128, MT2], F32, kind="ExternalOutput")
    with tile.TileContext(nc) as tc:
        _build_body(tc, x1t, x2t, dist1, dist2p, repeat)
    nc.compile()
    return nc


_NC_CACHE = {}


def get_nc(repeat=1):
    key = repeat
    if key not in _NC_CACHE:
        _NC_CACHE[key] = build_nc(repeat)
    return _NC_CACHE[key]


def make_in_maps(xyz1, xyz2):
    in_maps = []
    for c in range(NCORES):
        b, h = divmod(c, 2)
        x1 = xyz1[b, h * NHALF : (h + 1) * NHALF, :]
        in_maps.append(
            {
                "x1t": np.ascontiguousarray(x1.T),
                "x2t": np.ascontiguousarray(xyz2[b].T),
            }
        )
    return in_maps


def combine(results):
    # Device outputs are clamped maxima of -d2: dist = -out (>= 0).
    s1 = 0.0
    s2 = 0.0
    for b in range(B):
        r0, r1 = results[2 * b], results[2 * b + 1]
        s1 += -r0["dist1"].T.reshape(-1).sum(dtype=np.float64)
        s1 += -r1["dist1"].T.reshape(-1).sum(dtype=np.float64)
        d2 = np.maximum(r0["dist2p"].T.reshape(-1), r1["dist2p"].T.reshape(-1))
        s2 += -d2.sum(dtype=np.float64)
    return np.float32(s1 / (B * N) + s2 / (B * M))


def kernel(xyz1, xyz2):
    xyz1 = np.asarray(xyz1, dtype=np.float32)
    xyz2 = np.asarray(xyz2, dtype=np.float32)
    nc = get_nc()
    res = run_bass_kernel_spmd(nc, make_in_maps(xyz1, xyz2), core_ids=list(range(NCORES)))
    return combine(res.results)


if __name__ == "__main__":
    rng = np.random.default_rng(0)
    a = rng.standard_normal((B, N, 3), dtype=np.float32)
    b = rng.standard_normal((B, M, 3), dtype=np.float32)
    print("kernel:", kernel(a, b))


# revision 7
# speedup vs baseline: 1.2533x; 1.2533x over previous
"""Chamfer distance (L2, squared) Bass kernel for Trainium2.

Problem: xyz1 (4, 8192, 3), xyz2 (4, 8192, 3) float32.
  d2[b, n, m] = ||xyz1[b,n] - xyz2[b,m]||^2
  out = mean_n(min_m d2) + mean_m(min_n d2)   (scalar, float32)

Sharding: 8 cores = (batch b in 0..3) x (half h of the N axis). Each core
computes, for its (b, h):
  - dist1[n] = min over all M of d2 for its 4096 rows (complete), and
  - partial dist2[m] = min over its 4096 rows (combined across the 2
    halves on the host with an elementwise min).

Device algorithm (per core):
  The PE emits NEGATED squared distances via a single K=16 bf16 matmul
  per tile using an exact-ish hi/lo decomposition
  (x = bf16(x) + bf16(x - bf16(x))):
    -d2 = (2 x1) . x2 - ||x1||^2 - ||x2||^2
  with features
    F1 = [y1h y1h y1l y1l n1h n1l 1 1]   (y1 = 2 x1, n1* = -||x1||^2 hi/lo)
    F2 = [x2h x2l x2h x2l 1 1 n2h n2l]   (n2* = -||x2||^2 hi/lo)
  so F1 . F2 accumulates all four cross products plus both (negated)
  norms in fp32 PSUM. The same two feature buffers serve as stationary
  or moving for the two output orientations (A: m on partitions ->
  dist2; B: n on partitions -> dist1).

  Reduction (min d2 == -max(-d2)): PSUM tiles are consumed in PAIRS by
  a runtime-registered custom DVE op PAIRMAX_ANT:
    out = max(Src0, Src1) elementwise; accum_out = max(seed, max(out))
  ScalarE evacuates tile A of each pair to SBUF fp16; the DVE PAIRMAX
  then streams tile B straight from PSUM (fp32) against the fp16 copy,
  max-accumulating into a [128,1] column with seed chaining across
  pairs. One DVE instruction consumes two tiles; there are no
  fold/merge chains and no epilogue reduce. Steady state per pair:
  PE 1.7us, ScalarE 2.0us, DVE 2.3us -> DVE-bound.

  fp16 only rounds tile-A values (~2.4e-4 rel, unbiased); measured
  output error vs the fp32 reference is ~1e-5..2e-4.
"""

import numpy as np

import concourse.bass as bass
import concourse.tile as tile
from concourse import bacc, mybir
from concourse.bass_utils import run_bass_kernel_spmd

B, N, M = 4, 8192, 8192
NCORES = 8
NHALF = N // 2  # 4096 xyz1 rows per core

F32 = mybir.dt.float32
BF16 = mybir.dt.bfloat16
FP16 = mybir.dt.float16
BIG = 3.0e38

MT2 = M // 128  # 64 stationary chunks, orientation A
NT1 = NHALF // 128  # 32 stationary chunks, orientation B
TW = 1024  # moving-tile width (2 PSUM banks)
TA = NHALF // TW  # 4 moving tiles, orientation A
TB = M // TW  # 8 moving tiles, orientation B
PA = TA // 2  # pairs per chunk, orientation A
PB = TB // 2  # pairs per chunk, orientation B

MULT = mybir.AluOpType.mult
SUB = mybir.AluOpType.subtract
BYP = mybir.AluOpType.bypass


def register_pairmax():
    """Register the PAIRMAX_ANT custom DVE op (idempotent).

    out = max(Src0, Src1); accum_out = max(accum_init(=s0), max(out)).
    uops_sha is computed at registration so the pinned-sha check in
    DveOp.compile passes without editing concourse.
    """
    from concourse.dve_spec import Spec, Src0, Src1, C0, maxx, lower
    from concourse.dve_uop import DveOpSpec
    from concourse import dve_ops
    from concourse.dve_ops import DveOp

    if "PAIRMAX_ANT" in dve_ops._SUB_OPCODE_FOR_NAME:
        return next(o for o in dve_ops.OPS if o.name == "PAIRMAX_ANT")

    spec = Spec(
        body=maxx(Src0, Src1),
        accum=maxx,
        accum_init=C0,
        reference=lambda in0, in1, s0, s1, imm2: (
            np.maximum(in0, in1),
            np.maximum(np.maximum(in0, in1).max(axis=-1), s0.reshape(-1)),
        ),
    )
    row = dve_ops._CUSTOM_DVE_ROW_BASE + len(dve_ops.OPS)
    shas = {}
    for ver in ("v3", "v4"):
        uops = lower(spec, ver=ver)
        shas[ver] = DveOpSpec(
            name="PAIRMAX_ANT", opcode=row, uops=uops, rd1_en=True
        ).sha(ver)
    op = DveOp("PAIRMAX_ANT", spec, subdim=False, uops_sha=shas)
    dve_ops.OPS.append(op)
    dve_ops._SUB_OPCODE_FOR_NAME[op.name] = row
    dve_ops.CUSTOM_DVE_SPECS[op.name] = op.spec
    return op


PAIRMAX = register_pairmax()


def _build_body(tc, x1t, x2t, dist1, dist2p, repeat):
    # Compute instructions may only start at partition 0/32/64/96 (BIR
    # verifier rule), so all feature rows are computed at partition base 0
    # and placed into their final partition rows via SBUF->SBUF DMAs
    # (DMAs are exempt from the rule).
    nc = tc.nc
    stt = nc.vector.scalar_tensor_tensor

    persist = tc.alloc_tile_pool(name="persist", bufs=1)
    prep = tc.alloc_tile_pool(name="prep", bufs=1)

    f1 = persist.tile([16, NHALF], BF16)
    f2 = persist.tile([16, M], BF16)
    d1col = persist.tile([128, NT1], F32)  # negated: max(-d2) per chunk
    d2col = persist.tile([128, MT2], F32)
    # Per-pair accumulator mini-columns (independent PAIRMAX outputs; no
    # seed chaining so DVE instructions never wait on each other).
    colA = persist.tile([128, MT2 * PA], F32)
    colB = persist.tile([128, NT1 * PB], F32)

    neg_st = persist.tile([3, 128], F32)
    nc.vector.memset(neg_st[:], -1.0)
    nc.gpsimd.memset(f1[:], 1.0)
    nc.gpsimd.memset(f2[:], 1.0)

    def build_features(xin, feat, width, scale, hi_dup_rows, lo_rows, nrm_rows):
        # Shared-tag scratch so the x1 and x2 phases reuse the same SBUF.
        st = prep.tile([3, width], F32, tag="st", name="st")
        sq = prep.tile([3, width], F32, tag="sq", name="sq")
        lo = prep.tile([3, width], BF16, tag="lo", name="lo")
        nh = prep.tile([1, width], BF16, tag="nh", name="nh")
        nl = prep.tile([1, width], BF16, tag="nl", name="nl")

        nc.sync.dma_start(st[:], xin.ap())
        # -||x||^2: fp32 squares (ScalarE), then a (-1)-stationary matmul
        # broadcasts the negated per-point norm onto all 128 PSUM
        # partitions; row 0 is split hi/lo straight out of PSUM.
        nc.scalar.activation(sq[:], st[:], mybir.ActivationFunctionType.Square)
        psn = tc.alloc_tile_pool(name="psn", bufs=2, space="PSUM")
        for c in range(width // 512):
            sl = slice(512 * c, 512 * (c + 1))
            pn = psn.tile([128, 512], F32, tag="pn", name="pn")
            nc.tensor.matmul(pn[:], neg_st[:], sq[:, sl], start=True, stop=True)
            nc.scalar.copy(nh[0:1, sl], pn[0:1, :])
            stt(nl[0:1, sl], pn[0:1, :], 0.0, nh[0:1, sl], BYP, SUB)
        psn.release()

        # hi/lo split of (scale * x) at partition base 0.
        nc.scalar.mul(feat[0:3, :], st[:], scale)  # hi -> rows 0-2
        stt(lo[:], st[:], scale, feat[0:3, :], MULT, SUB)

        # Place remaining rows (DMAs may start at any partition).
        nc.sync.dma_start(feat[hi_dup_rows[0] : hi_dup_rows[0] + 3, :], feat[0:3, :])
        for r in lo_rows:
            nc.sync.dma_start(feat[r : r + 3, :], lo[:])
        nc.sync.dma_start(feat[nrm_rows[0] : nrm_rows[0] + 1, :], nh[:])
        nc.sync.dma_start(feat[nrm_rows[1] : nrm_rows[1] + 1, :], nl[:])

    # F1 = [y1h y1h y1l y1l n1h n1l 1 1]   (y1 = 2 x1)
    build_features(x1t, f1, NHALF, 2.0, (3,), (6, 9), (12, 13))
    # F2 = [x2h x2l x2h x2l 1 1 n2h n2l]
    build_features(x2t, f2, M, 1.0, (6,), (3, 9), (14, 15))

    prep.release()

    aux = tc.alloc_tile_pool(name="aux", bufs=1)
    pool_a = tc.alloc_tile_pool(name="ps_a", bufs=2, space="PSUM")
    pool_b = tc.alloc_tile_pool(name="ps_b", bufs=2, space="PSUM")

    def sweep(stat, mov, n_stat, n_pairs, colbuf):
        # One output orientation: for each 128-wide stationary chunk,
        # stream moving tiles of TW in pairs. Tile A of each pair is
        # evacuated to SBUF fp16 by ScalarE; tile B is consumed by one
        # PAIRMAX straight from PSUM, max-accumulating both tiles into an
        # independent mini-column of colbuf (reduced across pairs at the
        # end of the pass).
        for s in range(n_stat):
            for p in range(n_pairs):
                psA = pool_a.tile([128, TW], F32, tag="psA", name="psA")
                for j in range(TW // 512):
                    c = (2 * p) * (TW // 512) + j
                    nc.tensor.matmul(
                        psA[:, 512 * j : 512 * (j + 1)],
                        stat[:, 128 * s : 128 * (s + 1)],
                        mov[:, 512 * c : 512 * (c + 1)],
                        start=True,
                        stop=True,
                    )
                gt = aux.tile([128, TW], FP16, tag="gt", name="gt", bufs=3)
                nc.scalar.copy(gt[:], psA[:])
                psB = pool_b.tile([128, TW], F32, tag="psB", name="psB")
                for j in range(TW // 512):
                    c = (2 * p + 1) * (TW // 512) + j
                    nc.tensor.matmul(
                        psB[:, 512 * j : 512 * (j + 1)],
                        stat[:, 128 * s : 128 * (s + 1)],
                        mov[:, 512 * c : 512 * (c + 1)],
                        start=True,
                        stop=True,
                    )
                sc = aux.tile([128, TW], FP16, tag="sc", name="sc", bufs=2)
                nc.vector._custom_dve(
                    PAIRMAX,
                    out=sc[:],
                    in0=psB[:],
                    in1=gt[:],
                    s0=-BIG,
                    accum_out=colbuf[:, s * n_pairs + p : s * n_pairs + p + 1],
                )

    def one_pass():
        sweep(f2, f1, MT2, PA, colA)  # A: m on partitions -> dist2
        sweep(f1, f2, NT1, PB, colB)  # B: n on partitions -> dist1
        nc.vector.tensor_reduce(
            d2col[:],
            colA[:].rearrange("p (s q) -> p s q", q=PA),
            axis=mybir.AxisListType.X,
            op=mybir.AluOpType.max,
        )
        nc.vector.tensor_reduce(
            d1col[:],
            colB[:].rearrange("p (s q) -> p s q", q=PB),
            axis=mybir.AxisListType.X,
            op=mybir.AluOpType.max,
        )

    if repeat == 1:
        one_pass()
    else:
        # Benchmarking mode: re-run the main loop on-device so its cost
        # dominates the fixed host/RPC dispatch overhead.
        with tc.For_i(0, repeat, 1):
            one_pass()

    pool_b.release()
    pool_a.release()
    aux.release()

    # Clamp: d2 >= 0  <=>  max(-d2) <= 0.
    nc.vector.tensor_scalar_min(d1col[:], d1col[:], 0.0)
    nc.vector.tensor_scalar_min(d2col[:], d2col[:], 0.0)
    nc.sync.dma_start(dist1.ap(), d1col[:])
    nc.sync.dma_start(dist2p.ap(), d2col[:])

    persist.release()


def build_nc(repeat=1):
    nc = bacc.Bacc(
        "TRN2", target_bir_lowering=False, debug=False, num_devices=NCORES
    )
    x1t = nc.dram_tensor("x1t", [3, NHALF], F32, kind="ExternalInput")
    x2t = nc.dram_tensor("x2t", [3, M], F32, kind="ExternalInput")
    dist1 = nc.dram_tensor("dist1", [128, NT1], F32, kind="ExternalOutput")
    dist2p = nc.dram_tensor("dist2p", [128, MT2], F32, kind="ExternalOutput")
    with tile.TileContext(nc) as tc:
        _build_body(tc, x1t, x2t, dist1, dist2p, repeat)
    nc.compile()
    return nc


_NC_CACHE = {}


def get_nc(repeat=1):
    key = repeat
    if key not in _NC_CACHE:
        _NC_CACHE[key] = build_nc(repeat)
    return _NC_CACHE[key]


def make_in_maps(xyz1, xyz2):
    in_maps = []
    for c in range(NCORES):
        b, h = divmod(c, 2)
        x1 = xyz1[b, h * NHALF : (h + 1) * NHALF, :]
        in_maps.append(
            {
                "x1t": np.ascontiguousarray(x1.T),
                "x2t": np.ascontiguousarray(xyz2[b].T),
            }
        )
    return in_maps


def combine(results):
    # Device outputs are clamped maxima of -d2: dist = -out (>= 0).
    s1 = 0.0
    s2 = 0.0
    for b in range(B):
        r0, r1 = results[2 * b], results[2 * b + 1]
        s1 += -r0["dist1"].T.reshape(-1).sum(dtype=np.float64)
        s1 += -r1["dist1"].T.reshape(-1).sum(dtype=np.float64)
        d2 = np.maximum(r0["dist2p"].T.reshape(-1), r1["dist2p"].T.reshape(-1))
        s2 += -d2.sum(dtype=np.float64)
    return np.float32(s1 / (B * N) + s2 / (B * M))


def kernel(xyz1, xyz2):
    xyz1 = np.asarray(xyz1, dtype=np.float32)
    xyz2 = np.asarray(xyz2, dtype=np.float32)
    nc = get_nc()
    res = run_bass_kernel_spmd(nc, make_in_maps(xyz1, xyz2), core_ids=list(range(NCORES)))
    return combine(res.results)


if __name__ == "__main__":
    rng = np.random.default_rng(0)
    a = rng.standard_normal((B, N, 3), dtype=np.float32)
    b = rng.standard_normal((B, M, 3), dtype=np.float32)
    print("kernel:", kernel(a, b))


# revision 10
# speedup vs baseline: 1.2831x; 1.0238x over previous
"""Chamfer distance (L2, squared) Bass kernel for Trainium2.

Problem: xyz1 (4, 8192, 3), xyz2 (4, 8192, 3) float32.
  d2[b, n, m] = ||xyz1[b,n] - xyz2[b,m]||^2
  out = mean_n(min_m d2) + mean_m(min_n d2)   (scalar, float32)

Sharding: 8 cores = (batch b in 0..3) x (half h of the N axis). Each core
computes, for its (b, h):
  - dist1[n] = min over all M of d2 for its 4096 rows (complete), and
  - partial dist2[m] = min over its 4096 rows (combined across the 2
    halves on the host with an elementwise min).

Device algorithm (per core):
  The PE emits NEGATED squared distances via a single K=16 bf16 matmul
  per tile using an exact-ish hi/lo decomposition
  (x = bf16(x) + bf16(x - bf16(x))):
    -d2 = (2 x1) . x2 - ||x1||^2 - ||x2||^2
  with features
    F1 = [y1h y1h y1l y1l n1h n1l 1 1]   (y1 = 2 x1, n1* = -||x1||^2 hi/lo)
    F2 = [x2h x2l x2h x2l 1 1 n2h n2l]   (n2* = -||x2||^2 hi/lo)
  so F1 . F2 accumulates all four cross products plus both (negated)
  norms in fp32 PSUM. The same two feature buffers serve as stationary
  or moving for the two output orientations (A: m on partitions ->
  dist2; B: n on partitions -> dist1).

  Reduction (min d2 == -max(-d2)): PSUM tiles are consumed in PAIRS by
  a runtime-registered custom DVE op PAIRMAX_ANT:
    out = max(Src0, Src1) elementwise; accum_out = max(seed, max(out))
  ScalarE evacuates tile A of each pair to SBUF fp16; the DVE PAIRMAX
  then streams tile B straight from PSUM (fp32) against the fp16 copy,
  max-accumulating into a [128,1] column with seed chaining across
  pairs. One DVE instruction consumes two tiles; there are no
  fold/merge chains and no epilogue reduce. Steady state per pair:
  PE 1.7us, ScalarE 2.0us, DVE 2.3us -> DVE-bound.

  fp16 only rounds tile-A values (~2.4e-4 rel, unbiased); measured
  output error vs the fp32 reference is ~1e-5..2e-4.
"""

import numpy as np

import concourse.bass as bass
import concourse.tile as tile
from concourse import bacc, mybir
from concourse.bass_utils import run_bass_kernel_spmd

B, N, M = 4, 8192, 8192
NCORES = 8
NHALF = N // 2  # 4096 xyz1 rows per core

F32 = mybir.dt.float32
BF16 = mybir.dt.bfloat16
FP16 = mybir.dt.float16
BIG = 3.0e38

MT2 = M // 128  # 64 stationary chunks, orientation A
NT1 = NHALF // 128  # 32 stationary chunks, orientation B
TW = 1024  # moving-tile width (2 PSUM banks)
TA = NHALF // TW  # 4 moving tiles, orientation A
TB = M // TW  # 8 moving tiles, orientation B
PA = TA // 2  # pairs per chunk, orientation A
PB = TB // 2  # pairs per chunk, orientation B

MULT = mybir.AluOpType.mult
SUB = mybir.AluOpType.subtract
BYP = mybir.AluOpType.bypass


def register_pairmax():
    """Register the PAIRMAX_ANT custom DVE op (idempotent).

    out = max(Src0, Src1); accum_out = max(accum_init(=s0), max(out)).
    uops_sha is computed at registration so the pinned-sha check in
    DveOp.compile passes without editing concourse.
    """
    from concourse.dve_spec import Spec, Src0, Src1, C0, maxx, lower
    from concourse.dve_uop import DveOpSpec
    from concourse import dve_ops
    from concourse.dve_ops import DveOp

    if "PAIRMAX_ANT" in dve_ops._SUB_OPCODE_FOR_NAME:
        return next(o for o in dve_ops.OPS if o.name == "PAIRMAX_ANT")

    spec = Spec(
        body=maxx(Src0, Src1),
        accum=maxx,
        accum_init=C0,
        reference=lambda in0, in1, s0, s1, imm2: (
            np.maximum(in0, in1),
            np.maximum(np.maximum(in0, in1).max(axis=-1), s0.reshape(-1)),
        ),
    )
    row = dve_ops._CUSTOM_DVE_ROW_BASE + len(dve_ops.OPS)
    shas = {}
    for ver in ("v3", "v4"):
        uops = lower(spec, ver=ver)
        shas[ver] = DveOpSpec(
            name="PAIRMAX_ANT", opcode=row, uops=uops, rd1_en=True
        ).sha(ver)
    op = DveOp("PAIRMAX_ANT", spec, subdim=False, uops_sha=shas)
    dve_ops.OPS.append(op)
    dve_ops._SUB_OPCODE_FOR_NAME[op.name] = row
    dve_ops.CUSTOM_DVE_SPECS[op.name] = op.spec
    return op


PAIRMAX = register_pairmax()


def _build_body(tc, x1t, x2t, dist1, dist2p, repeat):
    # Compute instructions may only start at partition 0/32/64/96 (BIR
    # verifier rule), so all feature rows are computed at partition base 0
    # and placed into their final partition rows via SBUF->SBUF DMAs
    # (DMAs are exempt from the rule).
    nc = tc.nc
    stt = nc.vector.scalar_tensor_tensor

    persist = tc.alloc_tile_pool(name="persist", bufs=1)
    prep = tc.alloc_tile_pool(name="prep", bufs=1)

    f1 = persist.tile([16, NHALF], BF16)
    f2 = persist.tile([16, M], BF16)
    d1col = persist.tile([128, NT1], F32)  # negated: max(-d2) per chunk
    d2col = persist.tile([128, MT2], F32)
    # Per-pair accumulator mini-columns (independent PAIRMAX outputs; no
    # seed chaining so DVE instructions never wait on each other).
    colA = persist.tile([128, MT2 * PA], F32)
    colB = persist.tile([128, NT1 * PB], F32)

    neg_st = persist.tile([3, 128], F32)
    nc.vector.memset(neg_st[:], -1.0)
    nc.gpsimd.memset(f1[:], 1.0)
    nc.gpsimd.memset(f2[:], 1.0)

    def build_features(xin, feat, width, scale, hi_dup_rows, lo_rows, nrm_rows):
        # Shared-tag scratch so the x1 and x2 phases reuse the same SBUF.
        st = prep.tile([3, width], F32, tag="st", name="st")
        sq = prep.tile([3, width], F32, tag="sq", name="sq")
        lo = prep.tile([3, width], BF16, tag="lo", name="lo")
        nh = prep.tile([1, width], BF16, tag="nh", name="nh")
        nl = prep.tile([1, width], BF16, tag="nl", name="nl")

        nc.sync.dma_start(st[:], xin.ap())
        # -||x||^2: fp32 squares (ScalarE), then a (-1)-stationary matmul
        # broadcasts the negated per-point norm onto all 128 PSUM
        # partitions; row 0 is split hi/lo straight out of PSUM.
        nc.scalar.activation(sq[:], st[:], mybir.ActivationFunctionType.Square)
        psn = tc.alloc_tile_pool(name="psn", bufs=2, space="PSUM")
        for c in range(width // 512):
            sl = slice(512 * c, 512 * (c + 1))
            pn = psn.tile([128, 512], F32, tag="pn", name="pn")
            nc.tensor.matmul(pn[:], neg_st[:], sq[:, sl], start=True, stop=True)
            nc.scalar.copy(nh[0:1, sl], pn[0:1, :])
            stt(nl[0:1, sl], pn[0:1, :], 0.0, nh[0:1, sl], BYP, SUB)
        psn.release()

        # hi/lo split of (scale * x) at partition base 0.
        nc.scalar.mul(feat[0:3, :], st[:], scale)  # hi -> rows 0-2
        stt(lo[:], st[:], scale, feat[0:3, :], MULT, SUB)

        # Place remaining rows (DMAs may start at any partition).
        nc.sync.dma_start(feat[hi_dup_rows[0] : hi_dup_rows[0] + 3, :], feat[0:3, :])
        for r in lo_rows:
            nc.sync.dma_start(feat[r : r + 3, :], lo[:])
        nc.sync.dma_start(feat[nrm_rows[0] : nrm_rows[0] + 1, :], nh[:])
        nc.sync.dma_start(feat[nrm_rows[1] : nrm_rows[1] + 1, :], nl[:])

    # F1 = [y1h y1h y1l y1l n1h n1l 1 1]   (y1 = 2 x1)
    build_features(x1t, f1, NHALF, 2.0, (3,), (6, 9), (12, 13))
    # F2 = [x2h x2l x2h x2l 1 1 n2h n2l]
    build_features(x2t, f2, M, 1.0, (6,), (3, 9), (14, 15))

    prep.release()

    aux = tc.alloc_tile_pool(name="aux", bufs=1)
    pool_a = tc.alloc_tile_pool(name="ps_a", bufs=2, space="PSUM")
    pool_b = tc.alloc_tile_pool(name="ps_b", bufs=2, space="PSUM")

    def sweep(stat, mov, n_stat, n_pairs, colbuf):
        # One output orientation: for each 128-wide stationary chunk,
        # stream moving tiles of TW in pairs. Tile A of each pair is
        # evacuated to SBUF fp16 by ScalarE; tile B is consumed by one
        # PAIRMAX straight from PSUM, max-accumulating both tiles into an
        # independent mini-column of colbuf (reduced across pairs at the
        # end of the pass).
        for s in range(n_stat):
            for p in range(n_pairs):
                psA = pool_a.tile([128, TW], F32, tag="psA", name="psA")
                for j in range(TW // 512):
                    c = (2 * p) * (TW // 512) + j
                    nc.tensor.matmul(
                        psA[:, 512 * j : 512 * (j + 1)],
                        stat[:, 128 * s : 128 * (s + 1)],
                        mov[:, 512 * c : 512 * (c + 1)],
                        start=True,
                        stop=True,
                    )
                gt = aux.tile([128, TW], FP16, tag="gt", name="gt", bufs=6)
                nc.scalar.copy(gt[:], psA[:])
                psB = pool_b.tile([128, TW], F32, tag="psB", name="psB")
                for j in range(TW // 512):
                    c = (2 * p + 1) * (TW // 512) + j
                    nc.tensor.matmul(
                        psB[:, 512 * j : 512 * (j + 1)],
                        stat[:, 128 * s : 128 * (s + 1)],
                        mov[:, 512 * c : 512 * (c + 1)],
                        start=True,
                        stop=True,
                    )
                sc = aux.tile([128, TW], FP16, tag="sc", name="sc", bufs=4)
                nc.vector._custom_dve(
                    PAIRMAX,
                    out=sc[:],
                    in0=psB[:],
                    in1=gt[:],
                    s0=-BIG,
                    accum_out=colbuf[:, s * n_pairs + p : s * n_pairs + p + 1],
                )

    def one_pass():
        sweep(f2, f1, MT2, PA, colA)  # A: m on partitions -> dist2
        sweep(f1, f2, NT1, PB, colB)  # B: n on partitions -> dist1
        nc.vector.tensor_reduce(
            d2col[:],
            colA[:].rearrange("p (s q) -> p s q", q=PA),
            axis=mybir.AxisListType.X,
            op=mybir.AluOpType.max,
        )
        nc.vector.tensor_reduce(
            d1col[:],
            colB[:].rearrange("p (s q) -> p s q", q=PB),
            axis=mybir.AxisListType.X,
            op=mybir.AluOpType.max,
        )

    if repeat == 1:
        one_pass()
    else:
        # Benchmarking mode: re-run the main loop on-device so its cost
        # dominates the fixed host/RPC dispatch overhead. Two passes per
        # For_i iteration halve the all-engine-barrier drain per rep.
        assert repeat % 2 == 0
        with tc.For_i(0, repeat // 2, 1):
            one_pass()
            one_pass()

    pool_b.release()
    pool_a.release()
    aux.release()

    # Clamp: d2 >= 0  <=>  max(-d2) <= 0.
    nc.vector.tensor_scalar_min(d1col[:], d1col[:], 0.0)
    nc.vector.tensor_scalar_min(d2col[:], d2col[:], 0.0)
    nc.sync.dma_start(dist1.ap(), d1col[:])
    nc.sync.dma_start(dist2p.ap(), d2col[:])

    persist.release()


def build_nc(repeat=1):
    nc = bacc.Bacc(
        "TRN2", target_bir_lowering=False, debug=False, num_devices=NCORES
    )
    x1t = nc.dram_tensor("x1t", [3, NHALF], F32, kind="ExternalInput")
    x2t = nc.dram_tensor("x2t", [3, M], F32, kind="ExternalInput")
    dist1 = nc.dram_tensor("dist1", [128, NT1], F32, kind="ExternalOutput")
    dist2p = nc.dram_tensor("dist2p", [128, MT2], F32, kind="ExternalOutput")
    with tile.TileContext(nc) as tc:
        _build_body(tc, x1t, x2t, dist1, dist2p, repeat)
    nc.compile()
    return nc


_NC_CACHE = {}


def get_nc(repeat=1):
    key = repeat
    if key not in _NC_CACHE:
        _NC_CACHE[key] = build_nc(repeat)
    return _NC_CACHE[key]


def make_in_maps(xyz1, xyz2):
    in_maps = []
    for c in range(NCORES):
        b, h = divmod(c, 2)
        x1 = xyz1[b, h * NHALF : (h + 1) * NHALF, :]
        in_maps.append(
            {
                "x1t": np.ascontiguousarray(x1.T),
                "x2t": np.ascontiguousarray(xyz2[b].T),
            }
        )
    return in_maps


def combine(results):
    # Device outputs are clamped maxima of -d2: dist = -out (>= 0).
    s1 = 0.0
    s2 = 0.0
    for b in range(B):
        r0, r1 = results[2 * b], results[2 * b + 1]
        s1 += -r0["dist1"].T.reshape(-1).sum(dtype=np.float64)
        s1 += -r1["dist1"].T.reshape(-1).sum(dtype=np.float64)
        d2 = np.maximum(r0["dist2p"].T.reshape(-1), r1["dist2p"].T.reshape(-1))
        s2 += -d2.sum(dtype=np.float64)
    return np.float32(s1 / (B * N) + s2 / (B * M))


def kernel(xyz1, xyz2):
    xyz1 = np.asarray(xyz1, dtype=np.float32)
    xyz2 = np.asarray(xyz2, dtype=np.float32)
    nc = get_nc()
    res = run_bass_kernel_spmd(nc, make_in_maps(xyz1, xyz2), core_ids=list(range(NCORES)))
    return combine(res.results)


if __name__ == "__main__":
    rng = np.random.default_rng(0)
    a = rng.standard_normal((B, N, 3), dtype=np.float32)
    b = rng.standard_normal((B, M, 3), dtype=np.float32)
    print("kernel:", kernel(a, b))


# revision 13
# speedup vs baseline: 1.3828x; 1.0777x over previous
"""Chamfer distance (L2, squared) Bass kernel for Trainium2.

Problem: xyz1 (4, 8192, 3), xyz2 (4, 8192, 3) float32.
  d2[b, n, m] = ||xyz1[b,n] - xyz2[b,m]||^2
  out = mean_n(min_m d2) + mean_m(min_n d2)   (scalar, float32)

Sharding: 8 cores = (batch b in 0..3) x (half h of the N axis). Each core
computes, for its (b, h):
  - dist1[n] = min over all M of d2 for its 4096 rows (complete), and
  - partial dist2[m] = min over its 4096 rows (combined across the 2
    halves on the host with an elementwise min).

Device algorithm (per core):
  The PE emits NEGATED squared distances via a single K=16 bf16 matmul
  per tile using an exact-ish hi/lo decomposition
  (x = bf16(x) + bf16(x - bf16(x))):
    -d2 = (2 x1) . x2 - ||x1||^2 - ||x2||^2
  with features
    F1 = [y1h y1h y1l y1l n1h n1l 1 1]   (y1 = 2 x1, n1* = -||x1||^2 hi/lo)
    F2 = [x2h x2l x2h x2l 1 1 n2h n2l]   (n2* = -||x2||^2 hi/lo)
  so F1 . F2 accumulates all four cross products plus both (negated)
  norms in fp32 PSUM. The same two feature buffers serve as stationary
  or moving for the two output orientations (A: m on partitions ->
  dist2; B: n on partitions -> dist1).

  Reduction (min d2 == -max(-d2)): PSUM tiles are consumed in PAIRS by
  a runtime-registered custom DVE op PAIRMAX_ANT:
    out = max(Src0, Src1) elementwise; accum_out = max(seed, max(out))
  ScalarE evacuates tile A of each pair to SBUF fp16; the DVE PAIRMAX
  then streams tile B straight from PSUM (fp32) against the fp16 copy,
  max-accumulating into a [128,1] column with seed chaining across
  pairs. One DVE instruction consumes two tiles; there are no
  fold/merge chains and no epilogue reduce. Steady state per pair:
  PE 1.7us, ScalarE 2.0us, DVE 2.3us -> DVE-bound.

  fp16 only rounds tile-A values (~2.4e-4 rel, unbiased); measured
  output error vs the fp32 reference is ~1e-5..2e-4.
"""

import numpy as np

import concourse.bass as bass
import concourse.tile as tile
from concourse import bacc, mybir
from concourse.bass_utils import run_bass_kernel_spmd

B, N, M = 4, 8192, 8192
NCORES = 8
NHALF = N // 2  # 4096 xyz1 rows per core

F32 = mybir.dt.float32
BF16 = mybir.dt.bfloat16
FP16 = mybir.dt.float16
BIG = 3.0e38

MT2 = M // 128  # 64 stationary chunks, orientation A
NT1 = NHALF // 128  # 32 stationary chunks, orientation B
TW = 1024  # moving-tile width (2 PSUM banks)
TA = NHALF // TW  # 4 moving tiles, orientation A
TB = M // TW  # 8 moving tiles, orientation B
PA = TA // 2  # pairs per chunk, orientation A
PB = TB // 2  # pairs per chunk, orientation B

SB = 2  # single-pass n-blocks (0 or 2): dist1 via DMA-transpose path

MULT = mybir.AluOpType.mult
SUB = mybir.AluOpType.subtract
BYP = mybir.AluOpType.bypass


def register_pairmax():
    """Register the PAIRMAX_ANT custom DVE op (idempotent).

    out = max(Src0, Src1); accum_out = max(accum_init(=s0), max(out)).
    uops_sha is computed at registration so the pinned-sha check in
    DveOp.compile passes without editing concourse.
    """
    from concourse.dve_spec import Spec, Src0, Src1, C0, maxx, lower
    from concourse.dve_uop import DveOpSpec
    from concourse import dve_ops
    from concourse.dve_ops import DveOp

    if "PAIRMAX_ANT" in dve_ops._SUB_OPCODE_FOR_NAME:
        return next(o for o in dve_ops.OPS if o.name == "PAIRMAX_ANT")

    spec = Spec(
        body=maxx(Src0, Src1),
        accum=maxx,
        accum_init=C0,
        reference=lambda in0, in1, s0, s1, imm2: (
            np.maximum(in0, in1),
            np.maximum(np.maximum(in0, in1).max(axis=-1), s0.reshape(-1)),
        ),
    )
    row = dve_ops._CUSTOM_DVE_ROW_BASE + len(dve_ops.OPS)
    shas = {}
    for ver in ("v3", "v4"):
        uops = lower(spec, ver=ver)
        shas[ver] = DveOpSpec(
            name="PAIRMAX_ANT", opcode=row, uops=uops, rd1_en=True
        ).sha(ver)
    op = DveOp("PAIRMAX_ANT", spec, subdim=False, uops_sha=shas)
    dve_ops.OPS.append(op)
    dve_ops._SUB_OPCODE_FOR_NAME[op.name] = row
    dve_ops.CUSTOM_DVE_SPECS[op.name] = op.spec
    return op


PAIRMAX = register_pairmax()


def _build_body(tc, x1t, x2t, dist1, dist2p, repeat, sb):
    # Compute instructions may only start at partition 0/32/64/96 (BIR
    # verifier rule), so all feature rows are computed at partition base 0
    # and placed into their final partition rows via SBUF->SBUF DMAs
    # (DMAs are exempt from the rule).
    nc = tc.nc
    stt = nc.vector.scalar_tensor_tensor

    persist = tc.alloc_tile_pool(name="persist", bufs=1)
    prep = tc.alloc_tile_pool(name="prep", bufs=1)

    f1 = persist.tile([16, NHALF], BF16)
    f2 = persist.tile([16, M], BF16)
    d1col = persist.tile([128, NT1], F32)  # negated: max(-d2) per chunk
    d2col = persist.tile([128, MT2], F32)
    # Per-pair accumulator mini-columns (independent PAIRMAX outputs; no
    # seed chaining so DVE instructions never wait on each other).
    colA = persist.tile([128, MT2 * PA], F32)
    colB = persist.tile([128, NT1 * PB], F32)
    if sb > 0:
        # TD[j][g]: transposed group buffer [128, 8 n-chunks, 4 m-slots, 128 m]
        TDs = [
            [
                persist.tile([128, 8, 4, 128], FP16, name=f"td_{j}_{g}")
                for g in range(2)
            ]
            for j in range(sb)
        ]
        colT = persist.tile([128, sb * 8 * 8], F32)

    neg_st = persist.tile([3, 128], F32)
    nc.vector.memset(neg_st[:], -1.0)
    nc.gpsimd.memset(f1[:], 1.0)
    nc.gpsimd.memset(f2[:], 1.0)

    def build_features(xin, feat, width, scale, hi_dup_rows, lo_rows, nrm_rows):
        # Shared-tag scratch so the x1 and x2 phases reuse the same SBUF.
        st = prep.tile([3, width], F32, tag="st", name="st")
        sq = prep.tile([3, width], F32, tag="sq", name="sq")
        lo = prep.tile([3, width], BF16, tag="lo", name="lo")
        nh = prep.tile([1, width], BF16, tag="nh", name="nh")
        nl = prep.tile([1, width], BF16, tag="nl", name="nl")

        nc.sync.dma_start(st[:], xin.ap())
        # -||x||^2: fp32 squares (ScalarE), then a (-1)-stationary matmul
        # broadcasts the negated per-point norm onto all 128 PSUM
        # partitions; row 0 is split hi/lo straight out of PSUM.
        nc.scalar.activation(sq[:], st[:], mybir.ActivationFunctionType.Square)
        psn = tc.alloc_tile_pool(name="psn", bufs=2, space="PSUM")
        for c in range(width // 512):
            sl = slice(512 * c, 512 * (c + 1))
            pn = psn.tile([128, 512], F32, tag="pn", name="pn")
            nc.tensor.matmul(pn[:], neg_st[:], sq[:, sl], start=True, stop=True)
            nc.scalar.copy(nh[0:1, sl], pn[0:1, :])
            stt(nl[0:1, sl], pn[0:1, :], 0.0, nh[0:1, sl], BYP, SUB)
        psn.release()

        # hi/lo split of (scale * x) at partition base 0.
        nc.scalar.mul(feat[0:3, :], st[:], scale)  # hi -> rows 0-2
        stt(lo[:], st[:], scale, feat[0:3, :], MULT, SUB)

        # Place remaining rows (DMAs may start at any partition).
        nc.sync.dma_start(feat[hi_dup_rows[0] : hi_dup_rows[0] + 3, :], feat[0:3, :])
        for r in lo_rows:
            nc.sync.dma_start(feat[r : r + 3, :], lo[:])
        nc.sync.dma_start(feat[nrm_rows[0] : nrm_rows[0] + 1, :], nh[:])
        nc.sync.dma_start(feat[nrm_rows[1] : nrm_rows[1] + 1, :], nl[:])

    # F1 = [y1h y1h y1l y1l n1h n1l 1 1]   (y1 = 2 x1)
    build_features(x1t, f1, NHALF, 2.0, (3,), (6, 9), (12, 13))
    # F2 = [x2h x2l x2h x2l 1 1 n2h n2l]
    build_features(x2t, f2, M, 1.0, (6,), (3, 9), (14, 15))

    prep.release()

    aux = tc.alloc_tile_pool(name="aux", bufs=1)
    pool_a = tc.alloc_tile_pool(name="ps_a", bufs=2, space="PSUM")
    pool_b = tc.alloc_tile_pool(name="ps_b", bufs=2, space="PSUM")

    # Single-pass n-blocks (SB of the 4): their dist1 columns come from
    # DMA-transposing the evac'd fp16 A-tiles instead of a second
    # (B-orientation) matmul sweep. Transposed groups of 4 m-chunks are
    # kept per S-block; pairs of groups are consumed by one PAIRMAX per
    # 128-n-chunk slice. Transposes ride the otherwise-idle DMA engines
    # (alternating SP/Activation issue queues).
    def mm_tile(ps, stat, mov, s, t):
        for j in range(TW // 512):
            c = t * (TW // 512) + j
            nc.tensor.matmul(
                ps[:, 512 * j : 512 * (j + 1)],
                stat[:, 128 * s : 128 * (s + 1)],
                mov[:, 512 * c : 512 * (c + 1)],
                start=True,
                stop=True,
            )

    def sweep_a():
        # Orientation A (m on partitions -> dist2). Pair p couples the
        # PSUM-consumed tile (n-block p, double region) with the evac'd
        # tile (n-block p + PA - SB... cross-pairing when SB>0 so every
        # evac'd tile is a single-pass tile needing transposition).
        for s in range(MT2):
            for p in range(PA):
                if sb == 0:
                    tE, tB = 2 * p, 2 * p + 1  # evac'd, psum-consumed
                    jS = None
                elif sb == 2:
                    tB, tE = p, p + 2  # D-blocks 0,1 / S-blocks 2,3
                    jS = p

                psA = pool_a.tile([128, TW], F32, tag="psA", name="psA")
                mm_tile(psA, f2, f1, s, tE)
                gt = aux.tile([128, TW], FP16, tag="gt", name="gt", bufs=6)
                nc.scalar.copy(gt[:], psA[:])
                psB = pool_b.tile([128, TW], F32, tag="psB", name="psB")
                mm_tile(psB, f2, f1, s, tB)
                sc = aux.tile([128, TW], FP16, tag="sc", name="sc", bufs=4)
                nc.vector._custom_dve(
                    PAIRMAX,
                    out=sc[:],
                    in0=psB[:],
                    in1=gt[:],
                    s0=-BIG,
                    accum_out=colA[:, s * PA + p : s * PA + p + 1],
                )
                if sb == 2:
                    td = TDs[jS][(s // 4) % 2]
                    eng = nc.sync if (s + p) % 2 == 0 else nc.scalar
                    eng.dma_start_transpose(td[:, :, s % 4, :], gt[:])
            if sb == 2 and s % 8 == 7:
                q = s // 8
                for j in range(sb):
                    for k in range(8):
                        scx = aux.tile(
                            [128, 512], FP16, tag="scx", name="scx", bufs=4
                        )
                        idx = (j * 8 + k) * 8 + q
                        nc.vector._custom_dve(
                            PAIRMAX,
                            out=scx[:].rearrange("p (a b) -> p a b", a=4),
                            in0=TDs[j][0][:, k, :, :],
                            in1=TDs[j][1][:, k, :, :],
                            s0=-BIG,
                            accum_out=colT[:, idx : idx + 1],
                        )

    def sweep_b():
        # Orientation B (n on partitions -> dist1) for double-region
        # n-chunks only.
        for s in range(NT1 - 8 * sb):
            for p in range(PB):
                psA = pool_a.tile([128, TW], F32, tag="psA", name="psA")
                mm_tile(psA, f1, f2, s, 2 * p)
                gt = aux.tile([128, TW], FP16, tag="gt", name="gt", bufs=6)
                nc.scalar.copy(gt[:], psA[:])
                psB = pool_b.tile([128, TW], F32, tag="psB", name="psB")
                mm_tile(psB, f1, f2, s, 2 * p + 1)
                sc = aux.tile([128, TW], FP16, tag="sc", name="sc", bufs=4)
                nc.vector._custom_dve(
                    PAIRMAX,
                    out=sc[:],
                    in0=psB[:],
                    in1=gt[:],
                    s0=-BIG,
                    accum_out=colB[:, s * PB + p : s * PB + p + 1],
                )

    def one_pass():
        sweep_a()
        sweep_b()
        nc.vector.tensor_reduce(
            d2col[:],
            colA[:].rearrange("p (s q) -> p s q", q=PA),
            axis=mybir.AxisListType.X,
            op=mybir.AluOpType.max,
        )
        nd = NT1 - 8 * sb
        if nd > 0:
            nc.vector.tensor_reduce(
                d1col[:, 0:nd],
                colB[:, 0 : nd * PB].rearrange("p (s q) -> p s q", q=PB),
                axis=mybir.AxisListType.X,
                op=mybir.AluOpType.max,
            )
        if sb > 0:
            nc.vector.tensor_reduce(
                d1col[:, nd:NT1],
                colT[:].rearrange("p (s q) -> p s q", q=8),
                axis=mybir.AxisListType.X,
                op=mybir.AluOpType.max,
            )

    if repeat == 1:
        one_pass()
    else:
        # Benchmarking mode: re-run the main loop on-device so its cost
        # dominates the fixed host/RPC dispatch overhead. Two passes per
        # For_i iteration halve the all-engine-barrier drain per rep.
        assert repeat % 2 == 0
        with tc.For_i(0, repeat // 2, 1):
            one_pass()
            one_pass()

    pool_b.release()
    pool_a.release()
    aux.release()

    # Clamp: d2 >= 0  <=>  max(-d2) <= 0.
    nc.vector.tensor_scalar_min(d1col[:], d1col[:], 0.0)
    nc.vector.tensor_scalar_min(d2col[:], d2col[:], 0.0)
    nc.sync.dma_start(dist1.ap(), d1col[:])
    nc.sync.dma_start(dist2p.ap(), d2col[:])

    persist.release()


def build_nc(repeat=1, sb=SB):
    nc = bacc.Bacc(
        "TRN2", target_bir_lowering=False, debug=False, num_devices=NCORES
    )
    x1t = nc.dram_tensor("x1t", [3, NHALF], F32, kind="ExternalInput")
    x2t = nc.dram_tensor("x2t", [3, M], F32, kind="ExternalInput")
    dist1 = nc.dram_tensor("dist1", [128, NT1], F32, kind="ExternalOutput")
    dist2p = nc.dram_tensor("dist2p", [128, MT2], F32, kind="ExternalOutput")
    with tile.TileContext(nc) as tc:
        _build_body(tc, x1t, x2t, dist1, dist2p, repeat, sb)
    nc.compile()
    return nc


_NC_CACHE = {}


def get_nc(repeat=1, sb=SB):
    key = (repeat, sb)
    if key not in _NC_CACHE:
        _NC_CACHE[key] = build_nc(repeat, sb)
    return _NC_CACHE[key]


def make_in_maps(xyz1, xyz2):
    in_maps = []
    for c in range(NCORES):
        b, h = divmod(c, 2)
        x1 = xyz1[b, h * NHALF : (h + 1) * NHALF, :]
        in_maps.append(
            {
                "x1t": np.ascontiguousarray(x1.T),
                "x2t": np.ascontiguousarray(xyz2[b].T),
            }
        )
    return in_maps


def combine(results):
    # Device outputs are clamped maxima of -d2: dist = -out (>= 0).
    s1 = 0.0
    s2 = 0.0
    for b in range(B):
        r0, r1 = results[2 * b], results[2 * b + 1]
        s1 += -r0["dist1"].T.reshape(-1).sum(dtype=np.float64)
        s1 += -r1["dist1"].T.reshape(-1).sum(dtype=np.float64)
        d2 = np.maximum(r0["dist2p"].T.reshape(-1), r1["dist2p"].T.reshape(-1))
        s2 += -d2.sum(dtype=np.float64)
    return np.float32(s1 / (B * N) + s2 / (B * M))


def kernel(xyz1, xyz2):
    xyz1 = np.asarray(xyz1, dtype=np.float32)
    xyz2 = np.asarray(xyz2, dtype=np.float32)
    nc = get_nc()
    res = run_bass_kernel_spmd(nc, make_in_maps(xyz1, xyz2), core_ids=list(range(NCORES)))
    return combine(res.results)


if __name__ == "__main__":
    rng = np.random.default_rng(0)
    a = rng.standard_normal((B, N, 3), dtype=np.float32)
    b = rng.standard_normal((B, M, 3), dtype=np.float32)
    print("kernel:", kernel(a, b))
